# revision 6
# baseline (speedup 1.0000x reference)
"""Trainium2 Bass kernel for nn_BG_LSTM: LSTM(input=1, hidden=256) over T=512,
batch 512, followed by ReLU + Linear(256, 1).

Sharding: data-parallel over batch across 8 cores (64 batch rows/core).
Weights replicated. The time recurrence runs locally per core.

Per-core layout ("folded"): every [64, 256] state tensor is stored as
[128, 128]: partition p<64 holds batch row p, hidden dims 0:128; partition
64+p holds batch row p, hidden dims 128:256.  The per-step gate matmul
produces a single PSUM tile G [128, 512] with column blocks
[i_fold | f_fold | o_fold | g_fold] via two concurrent column-group matmul
chains (tile_position (0,0) and (0,64)), so sigmoid covers G[:,0:384] in one
ACT instruction and tanh covers G[:,384:512] in another.  x_t and the bias
enter the accumulation as a K=2 matmul whose stationary [2, 64] tile
(row0 = x[:, t], row1 = ones) is DMA'd from DRAM each step (prefetched).
h is re-transposed for the next step's stationary with one PE transpose.
"""

import sys

sys.path.insert(0, "/opt/trn_rl_repo")

import numpy as np
from contextlib import ExitStack

import concourse.bass as bass
import concourse.bacc as bacc
import concourse.mybir as mybir
from concourse.tile import TileContext
from concourse.bass_utils import run_bass_kernel_spmd

B, T, H = 512, 512, 256
NCORES = 8
BL = B // NCORES  # 64 batch rows per core
G4 = 4 * H  # 1024
DT = mybir.dt.float32
AF = mybir.ActivationFunctionType

_k = np.arange(128)
# PyTorch gate row order: i(0:256), f(256:512), g(512:768), o(768:1024).
# Folded column order per group: [i, f, o, g] halves.
PERM_A = np.concatenate([0 + _k, 256 + _k, 768 + _k, 512 + _k])  # lo halves
PERM_B = np.concatenate([128 + _k, 384 + _k, 896 + _k, 640 + _k])  # hi halves

_CACHE = {}


# Packed constant column offsets in the single consts tile [128, CW]
_WA0, _WA1, _WB0, _WB1 = 0, 512, 1024, 1536
_ID = 2048
_WXA, _WXB = 2176, 2688
_WFC = 3200  # 2 cols
_BFC = 3202  # 1 col (rows 0:64)
CW = 3204


def _build(t_steps: int):
    nc = bacc.Bacc("TRN2", target_bir_lowering=False)
    p_xstep = nc.declare_dram_parameter("xstep", [t_steps, 2, BL], DT, isOutput=False)
    p_consts = nc.declare_dram_parameter("consts", [128, CW], DT, isOutput=False)
    p_out = nc.declare_dram_parameter("out", [BL, 1], DT, isOutput=True)

    with ExitStack() as ctx:
        tc = ctx.enter_context(TileContext(nc))
        cpool = ctx.enter_context(tc.tile_pool(name="consts", bufs=1))
        spool = ctx.enter_context(tc.tile_pool(name="state", bufs=1))
        xpool = ctx.enter_context(tc.tile_pool(name="xcur", bufs=8))
        wpool = ctx.enter_context(tc.tile_pool(name="work", bufs=2))
        gpool = ctx.enter_context(tc.tile_pool(name="gpsum", bufs=2, space="PSUM"))
        tpool = ctx.enter_context(tc.tile_pool(name="tpsum", bufs=2, space="PSUM"))
        fpool = ctx.enter_context(tc.tile_pool(name="fpsum", bufs=1, space="PSUM"))

        # One DMA for every constant => a single DMA-queue semaphore.
        cs = cpool.tile([128, CW], DT)
        nc.sync.dma_start(cs[:], p_consts[:])
        wa0, wa1 = cs[:, _WA0:_WA0 + 512], cs[:, _WA1:_WA1 + 512]
        wb0, wb1 = cs[:, _WB0:_WB0 + 512], cs[:, _WB1:_WB1 + 512]
        ident = cs[:, _ID:_ID + 128]
        wxa = cs[0:2, _WXA:_WXA + 512]
        wxb = cs[0:2, _WXB:_WXB + 512]
        wfc0, wfc1 = cs[:, _WFC:_WFC + 1], cs[:, _WFC + 1:_WFC + 2]
        bfc = cs[0:BL, _BFC:_BFC + 1]

        # Absorber: a tiny PE op that waits on the consts DMA so later
        # Matmults never need a DMA wait (walrus allows 1 sync-wait each).
        absb = tpool.tile([32, 32], DT, tag="absb")
        nc.tensor.transpose(absb[:], cs[0:32, _ID:_ID + 32], cs[0:32, _ID:_ID + 32])

        # Persistent state, zeroed on ScalarE (ACT) so the first matmuls
        # wait on the ACT semaphore only.
        c_fold = spool.tile([128, 128], DT)  # folded cell state
        tsb = spool.tile([128, 128], DT)  # transposed h (hT folded)
        nc.scalar.mul(c_fold[:], ident, 0.0)
        nc.scalar.mul(tsb[:], ident, 0.0)

        for t in range(t_steps):
            xcur = xpool.tile([2, BL], DT, tag="xcur")
            nc.sync.dma_start(xcur[:], p_xstep[t])

            g = gpool.tile([128, 512], DT, tag="g")
            # Group A -> partitions 0:64 (lo halves of all gates)
            nc.tensor.matmul(g[0:64, :], tsb[:, 0:64], wa0,
                             start=True, stop=False, tile_position=(0, 0))
            nc.tensor.matmul(g[0:64, :], tsb[:, 64:128], wa1,
                             start=False, stop=False, tile_position=(0, 0))
            nc.tensor.matmul(g[0:64, :], xcur[:], wxa,
                             start=False, stop=True, tile_position=(0, 0))
            # Group B -> partitions 64:128 (hi halves)
            nc.tensor.matmul(g[64:128, :], tsb[:, 0:64], wb0,
                             start=True, stop=False, tile_position=(0, 64))
            nc.tensor.matmul(g[64:128, :], tsb[:, 64:128], wb1,
                             start=False, stop=False, tile_position=(0, 64))
            nc.tensor.matmul(g[64:128, :], xcur[:], wxb,
                             start=False, stop=True, tile_position=(0, 64))

            # Nonlinearities: sigmoid(i,f,o), tanh(g)
            sifo = wpool.tile([128, 384], DT, tag="sifo")
            nc.scalar.activation(sifo[:], g[:, 0:384], AF.Sigmoid)
            tg = wpool.tile([128, 128], DT, tag="tg")
            nc.scalar.activation(tg[:], g[:, 384:512], AF.Tanh)

            # c = f*c + i*g
            u = wpool.tile([128, 128], DT, tag="u")
            nc.vector.tensor_mul(u[:], sifo[:, 128:256], c_fold[:])
            v = wpool.tile([128, 128], DT, tag="v")
            nc.vector.tensor_mul(v[:], sifo[:, 0:128], tg[:])
            nc.vector.tensor_add(c_fold[:], u[:], v[:])

            tcell = wpool.tile([128, 128], DT, tag="tcell")
            nc.scalar.activation(tcell[:], c_fold[:], AF.Tanh)

            hf = wpool.tile([128, 128], DT, tag="hf")
            nc.vector.tensor_mul(hf[:], sifo[:, 256:384], tcell[:])

            # Transpose h for next step's stationary.  The PSUM->SBUF copy
            # runs on ScalarE so the next step's matmuls wait on a single
            # engine semaphore (ACT) -- walrus caps sync-waits per Matmult.
            tp = tpool.tile([128, 128], DT, tag="tp")
            nc.tensor.transpose(tp[:], hf[:], ident)
            nc.scalar.copy(tsb[:], tp[:])

        # FC head: relu(h) @ W_fc.T + b_fc
        rl = wpool.tile([128, 128], DT, tag="rl")
        nc.scalar.activation(rl[:], tsb[:], AF.Relu)
        fc = fpool.tile([BL, 1], DT)
        nc.tensor.matmul(fc[:], rl[:, 0:64], wfc0, start=True, stop=False)
        nc.tensor.matmul(fc[:], rl[:, 64:128], wfc1, start=False, stop=True)
        ob = wpool.tile([BL, 1], DT, tag="ob")
        nc.vector.tensor_scalar_add(ob[:], fc[:], bfc)
        nc.sync.dma_start(p_out[:], ob[:])

    nc.compile()
    return nc


def _prep_inputs(x, W_ih, W_hh, b_ih, b_hh, W_fc, b_fc, t_steps):
    x = np.ascontiguousarray(np.asarray(x, dtype=np.float32))
    W_ih = np.asarray(W_ih, dtype=np.float32)
    W_hh = np.asarray(W_hh, dtype=np.float32)
    b = np.asarray(b_ih, dtype=np.float32) + np.asarray(b_hh, dtype=np.float32)
    W_fc = np.asarray(W_fc, dtype=np.float32)
    b_fc = np.asarray(b_fc, dtype=np.float32)

    WT = np.ascontiguousarray(W_hh.T)  # [256, 1024]
    WA = WT[:, PERM_A]  # [256, 512]
    WB = WT[:, PERM_B]
    cs = np.zeros((128, CW), dtype=np.float32)
    cs[:, _WA0:_WA0 + 512] = WA[0:128]
    cs[:, _WA1:_WA1 + 512] = WA[128:256]
    cs[:, _WB0:_WB0 + 512] = WB[0:128]
    cs[:, _WB1:_WB1 + 512] = WB[128:256]
    cs[:, _ID:_ID + 128] = np.eye(128, dtype=np.float32)
    cs[0:2, _WXA:_WXA + 512] = np.stack([W_ih[PERM_A, 0], b[PERM_A]])
    cs[0:2, _WXB:_WXB + 512] = np.stack([W_ih[PERM_B, 0], b[PERM_B]])
    cs[:, _WFC] = W_fc[0, 0:128]
    cs[:, _WFC + 1] = W_fc[0, 128:256]
    cs[0:BL, _BFC] = float(b_fc[0])
    shared = {"consts": cs}
    in_maps = []
    for c in range(NCORES):
        xs = x[c * BL:(c + 1) * BL, :t_steps]  # [64, T]
        xstep = np.empty((t_steps, 2, BL), dtype=np.float32)
        xstep[:, 0, :] = xs.T
        xstep[:, 1, :] = 1.0
        m = dict(shared)
        m["xstep"] = np.ascontiguousarray(xstep)
        in_maps.append(m)
    return in_maps


def _run(inputs, t_steps, trace=False):
    if t_steps not in _CACHE:
        _CACHE[t_steps] = _build(t_steps)
    nc = _CACHE[t_steps]
    in_maps = _prep_inputs(
        inputs["x"], inputs["W_ih"], inputs["W_hh"], inputs["b_ih"],
        inputs["b_hh"], inputs["W_fc"], inputs["b_fc"], t_steps,
    )
    kw = {}
    if trace:
        kw = dict(trace=True)
    res = run_bass_kernel_spmd(nc, in_maps, core_ids=list(range(NCORES)), **kw)
    out = np.concatenate([res.results[c]["out"] for c in range(NCORES)], axis=0)
    return out.astype(np.float32), res


def kernel(x, W_ih, W_hh, b_ih, b_hh, W_fc, b_fc):
    out, _ = _run(
        dict(x=x, W_ih=W_ih, W_hh=W_hh, b_ih=b_ih, b_hh=b_hh,
             W_fc=W_fc, b_fc=b_fc),
        T,
    )
    return out


# revision 17
# speedup vs baseline: 2.4829x; 2.4829x over previous
"""Trainium2 Bass kernel for nn_BG_LSTM: LSTM(input=1, hidden=256) over T=512,
batch 512, followed by ReLU + Linear(256, 1).

Sharding: data-parallel over batch across 8 cores (64 batch rows/core).
Weights replicated. The time recurrence runs locally per core.

Per-core layout ("folded"): every [64, 256] state tensor is stored as
[128, 128]: partition p<64 holds batch row p, hidden dims 0:128; partition
64+p holds batch row p, hidden dims 128:256.  The per-step gate matmul
produces a single PSUM tile G [128, 512] with column blocks
[i_fold | f_fold | o_fold | g_fold] via two concurrent column-group matmul
chains (tile_position (0,0) and (0,64)), so sigmoid covers G[:,0:384] in one
ACT instruction and tanh covers G[:,384:512] in another.  x_t and the bias
enter the accumulation as a K=2 matmul whose stationary [2, 64] tile
(row0 = x[:, t], row1 = ones) is DMA'd from DRAM each step (prefetched).
h is re-transposed for the next step's stationary with one PE transpose.
"""

import sys

sys.path.insert(0, "/opt/trn_rl_repo")

import numpy as np
from contextlib import ExitStack

import concourse.bass as bass
import concourse.bacc as bacc
import concourse.mybir as mybir
from concourse.tile import TileContext
from concourse.bass_utils import run_bass_kernel_spmd

B, T, H = 512, 512, 256
NCORES = 8
BL = B // NCORES  # 64 batch rows per core
G4 = 4 * H  # 1024
DT = mybir.dt.float32
AF = mybir.ActivationFunctionType

_k = np.arange(128)
# PyTorch gate row order: i(0:256), f(256:512), g(512:768), o(768:1024).
# Folded column order per group: [i, f, o, g] halves.
PERM_A = np.concatenate([0 + _k, 256 + _k, 768 + _k, 512 + _k])  # lo halves
PERM_B = np.concatenate([128 + _k, 384 + _k, 896 + _k, 640 + _k])  # hi halves

_CACHE = {}


# Packed constant column offsets in the single consts tile [128, CW]
_WA0, _WA1, _WB0, _WB1 = 0, 512, 1024, 1536
_ID = 2048
_WXA, _WXB = 2176, 2688
_WFC = 3200  # 2 cols
_BFC = 3202  # 1 col (rows 0:64)
CW = 3204


def _build(t_steps: int):
    nc = bacc.Bacc("TRN2", target_bir_lowering=False)
    p_xstep = nc.declare_dram_parameter("xstep", [T, 2, BL], DT, isOutput=False)
    p_consts = nc.declare_dram_parameter("consts", [128, CW], DT, isOutput=False)
    p_out = nc.declare_dram_parameter("out", [BL, 1], DT, isOutput=True)

    with ExitStack() as ctx:
        tc = ctx.enter_context(TileContext(nc))
        cpool = ctx.enter_context(tc.tile_pool(name="consts", bufs=1))
        spool = ctx.enter_context(tc.tile_pool(name="state", bufs=1))
        xpool = ctx.enter_context(tc.tile_pool(name="xcur", bufs=12))
        wpool = ctx.enter_context(tc.tile_pool(name="work", bufs=3))
        gpool = ctx.enter_context(tc.tile_pool(name="gpsum", bufs=3, space="PSUM"))
        tpool = ctx.enter_context(tc.tile_pool(name="tpsum", bufs=2, space="PSUM"))
        fpool = ctx.enter_context(tc.tile_pool(name="fpsum", bufs=1, space="PSUM"))

        # One DMA for every constant => a single DMA-queue semaphore.
        cs = cpool.tile([128, CW], DT)
        nc.sync.dma_start(cs[:], p_consts[:])
        wa0, wa1 = cs[:, _WA0:_WA0 + 512], cs[:, _WA1:_WA1 + 512]
        wb0, wb1 = cs[:, _WB0:_WB0 + 512], cs[:, _WB1:_WB1 + 512]
        ident = cs[:, _ID:_ID + 128]
        wxa = cs[0:2, _WXA:_WXA + 512]
        wxb = cs[0:2, _WXB:_WXB + 512]
        wfc0, wfc1 = cs[:, _WFC:_WFC + 1], cs[:, _WFC + 1:_WFC + 2]
        bfc = cs[0:BL, _BFC:_BFC + 1]

        # Absorber: a tiny PE op that waits on the consts DMA so later
        # Matmults never need a DMA wait (walrus allows 1 sync-wait each).
        absb = fpool.tile([32, 32], DT, tag="absb")
        nc.tensor.transpose(absb[:], cs[0:32, _ID:_ID + 32], cs[0:32, _ID:_ID + 32])

        # Persistent state, zeroed on ScalarE (ACT) so the first matmuls
        # wait on the ACT semaphore only.
        c_fold = spool.tile([128, 128], DT)  # folded cell state
        tsb = spool.tile([128, 128], DT)  # transposed h (hT folded)
        nc.scalar.mul(c_fold[:], ident, 0.0)
        nc.scalar.mul(tsb[:], ident, 0.0)

        for t in range(t_steps):
            xcur = xpool.tile([2, BL], DT, tag="xcur")
            nc.sync.dma_start(xcur[:], p_xstep[t])

            g = gpool.tile([128, 512], DT, tag="g")
            # x/bias matmuls first (start=True): they depend only on the
            # prefetched xcur DMA and the freed PSUM slot, so the PE can run
            # them while the previous step's elementwise tail is still going.
            # The weight matmuls (which need tsb) then finish the group.
            nc.tensor.matmul(g[0:64, :], xcur[:], wxa,
                             start=True, stop=False, tile_position=(0, 0), skip_group_check=True)
            nc.tensor.matmul(g[64:128, :], xcur[:], wxb,
                             start=True, stop=False, tile_position=(0, 64), skip_group_check=True)
            # Group A -> partitions 0:64 (lo halves of all gates)
            nc.tensor.matmul(g[0:64, :], tsb[:, 0:64], wa0,
                             start=False, stop=False, tile_position=(0, 0), skip_group_check=True)
            nc.tensor.matmul(g[0:64, :], tsb[:, 64:128], wa1,
                             start=False, stop=True, tile_position=(0, 0), skip_group_check=True)
            # Group B -> partitions 64:128 (hi halves)
            nc.tensor.matmul(g[64:128, :], tsb[:, 0:64], wb0,
                             start=False, stop=False, tile_position=(0, 64), skip_group_check=True)
            nc.tensor.matmul(g[64:128, :], tsb[:, 64:128], wb1,
                             start=False, stop=True, tile_position=(0, 64), skip_group_check=True)

            # All four gates in ONE ACT instruction: sigmoid(z) =
            # 0.5*(1+tanh(z/2)), with the /2 for i,f,o pre-scaled into the
            # weights host-side.  ta = [ti* | tf* | to* | tg] where
            # t•* = tanh(z/2) = 2*sigmoid(z)-1.
            ta = wpool.tile([128, 512], DT, tag="ta")
            nc.scalar.activation(ta[:], g[:, 0:512], AF.Tanh)

            # State S = 2c.  u = (1+tf*)*S = 4*sig(f)*c;  v = (1+ti*)*tg
            # = 2*sig(i)*tg;  S' = 0.5*u + v = 2c'.
            u = wpool.tile([128, 128], DT, tag="u")
            nc.vector.scalar_tensor_tensor(
                u[:], ta[:, 128:256], 1.0, c_fold[:],
                mybir.AluOpType.add, mybir.AluOpType.mult)
            v = wpool.tile([128, 128], DT, tag="v")
            nc.vector.scalar_tensor_tensor(
                v[:], ta[:, 0:128], 1.0, ta[:, 384:512],
                mybir.AluOpType.add, mybir.AluOpType.mult)
            nc.vector.scalar_tensor_tensor(
                c_fold[:], u[:], 0.5, v[:],
                mybir.AluOpType.mult, mybir.AluOpType.add)

            # tanh(c) = tanh(S/2) via ACT's free input scale.
            tcell = wpool.tile([128, 128], DT, tag="tcell")
            nc.scalar.activation(tcell[:], c_fold[:], AF.Tanh, scale=0.5)

            # hf = (1+to*)*tanh(c) = 2h; the extra 2 is folded into the
            # W_hh/W_fc columns host-side.
            hf = wpool.tile([128, 128], DT, tag="hf")
            nc.vector.scalar_tensor_tensor(
                hf[:], ta[:, 256:384], 1.0, tcell[:],
                mybir.AluOpType.add, mybir.AluOpType.mult)

            # Transpose h for next step's stationary (PSUM->SBUF via DVE,
            # keeping the busier ACT engine free).
            tp = tpool.tile([128, 128], DT, tag="tp")
            nc.tensor.transpose(tp[:], hf[:], ident)
            nc.vector.tensor_copy(tsb[:], tp[:])

        # FC head: relu(h) @ W_fc.T + b_fc
        rl = wpool.tile([128, 128], DT, tag="rl")
        nc.scalar.activation(rl[:], tsb[:], AF.Relu)
        fc = fpool.tile([BL, 1], DT)
        nc.tensor.matmul(fc[:], rl[:, 0:64], wfc0, start=True, stop=False)
        nc.tensor.matmul(fc[:], rl[:, 64:128], wfc1, start=False, stop=True)
        ob = wpool.tile([BL, 1], DT, tag="ob")
        nc.vector.tensor_scalar_add(ob[:], fc[:], bfc)
        nc.sync.dma_start(p_out[:], ob[:])

    nc.compile()
    return nc


def _prep_inputs(x, W_ih, W_hh, b_ih, b_hh, W_fc, b_fc, t_steps):
    x = np.ascontiguousarray(np.asarray(x, dtype=np.float32))
    W_ih = np.asarray(W_ih, dtype=np.float32)
    W_hh = np.asarray(W_hh, dtype=np.float32)
    b = np.asarray(b_ih, dtype=np.float32) + np.asarray(b_hh, dtype=np.float32)
    W_fc = np.asarray(W_fc, dtype=np.float32)
    b_fc = np.asarray(b_fc, dtype=np.float32)

    WT = np.ascontiguousarray(W_hh.T)  # [256, 1024]
    WA = WT[:, PERM_A]  # [256, 512]
    WB = WT[:, PERM_B]
    # Column scale for the all-tanh trick: sigmoid gates (i,f,o = cols
    # 0:384) get z/2 pre-scaling; every W_hh column gets an extra 0.5
    # because the stored h-state is 2h.
    gsc = np.ones(512, dtype=np.float32)
    gsc[0:384] = 0.5
    wsc = 0.5 * gsc  # for WA/WB (the h-operand side)
    cs = np.zeros((128, CW), dtype=np.float32)
    cs[:, _WA0:_WA0 + 512] = WA[0:128] * wsc
    cs[:, _WA1:_WA1 + 512] = WA[128:256] * wsc
    cs[:, _WB0:_WB0 + 512] = WB[0:128] * wsc
    cs[:, _WB1:_WB1 + 512] = WB[128:256] * wsc
    cs[:, _ID:_ID + 128] = np.eye(128, dtype=np.float32)
    cs[0:2, _WXA:_WXA + 512] = np.stack([W_ih[PERM_A, 0], b[PERM_A]]) * gsc
    cs[0:2, _WXB:_WXB + 512] = np.stack([W_ih[PERM_B, 0], b[PERM_B]]) * gsc
    cs[:, _WFC] = W_fc[0, 0:128] * 0.5
    cs[:, _WFC + 1] = W_fc[0, 128:256] * 0.5
    cs[0:BL, _BFC] = float(b_fc[0])
    shared = {"consts": cs}
    in_maps = []
    for c in range(NCORES):
        xs = x[c * BL:(c + 1) * BL, :]  # [64, T]
        xstep = np.empty((T, 2, BL), dtype=np.float32)
        xstep[:, 0, :] = xs.T
        xstep[:, 1, :] = 1.0
        m = dict(shared)
        m["xstep"] = np.ascontiguousarray(xstep)
        in_maps.append(m)
    return in_maps


def _run(inputs, t_steps, trace=False):
    if t_steps not in _CACHE:
        _CACHE[t_steps] = _build(t_steps)
    nc = _CACHE[t_steps]
    in_maps = _prep_inputs(
        inputs["x"], inputs["W_ih"], inputs["W_hh"], inputs["b_ih"],
        inputs["b_hh"], inputs["W_fc"], inputs["b_fc"], t_steps,
    )
    kw = {}
    if trace:
        kw = dict(trace=True)
    try:
        res = run_bass_kernel_spmd(nc, in_maps, core_ids=list(range(NCORES)), **kw)
    except ModuleNotFoundError:
        # NTFF profile hook unavailable (no antenv) -- rerun without trace.
        res = run_bass_kernel_spmd(nc, in_maps, core_ids=list(range(NCORES)))
    out = np.concatenate([res.results[c]["out"] for c in range(NCORES)], axis=0)
    return out.astype(np.float32), res


def kernel(x, W_ih, W_hh, b_ih, b_hh, W_fc, b_fc):
    out, _ = _run(
        dict(x=x, W_ih=W_ih, W_hh=W_hh, b_ih=b_ih, b_hh=b_hh,
             W_fc=W_fc, b_fc=b_fc),
        T,
    )
    return out
